# revision 9
# baseline (speedup 1.0000x reference)
"""Trainium2 kernel for the conditional optimal diffusion score
(per-class masked-softmax RBF regression over the dataset).

Math (see reference): for query u, dataset x (N,D), labels y (N,):
    logit_n = -(0.5/sigma2) * ||u - s*x_n||^2,  s = sqrt(alpha_bar[t])
            = -(s^2/(2*sigma2)) * ||x_n - u/s||^2
so ranking samples by logit (descending) == ranking by
    q_n = ||x_n - c||^2,  c = u/s   (ascending).
The per-class softmax at this noise level is extremely concentrated
(logit std across samples ~17), so the exact score is a tiny weighted
sum over the few nearest neighbours per class.  The device therefore
only needs q_n to ~1-logit accuracy for CANDIDATE SELECTION; the host
re-ranks the top-64 rows per class exactly in fp64.

Device strategy (per core, shard = 6250 rows of x):
  x is streamed TRANSPOSED (partitions = feature dim d, free = sample n)
  in fp8 e3m4 (1 byte/elem -> 4x less HBM traffic than fp32).  For each
  128-row feature chunk ct (24 per core):
    ScalarE chunks: sq = Square(x + b),   b = -c  (bias is per-partition)
    VectorE chunks: sq = (x + b2) * x,    b2 = -2c  (fused stt; differs
       from Square chunks only by a per-chunk constant sum(c_d^2), which
       is sample-independent and thus ranking-safe)
  and the 128-partition reduction q += ones^T @ sq runs on the otherwise
  idle PE array into PSUM (ones is a [128,1] stationary -> ~1 cycle
  weight load, 1 cycle/row streaming).
  The 6250 sample columns are processed in 2 halves of 3125 so the
  per-half PSUM accumulators (7 banks of [1,512] fp32) fit.

Engine budget per core: DMA 19.2MB fp8 ~54us, ScalarE 13 chunks ~68us,
VectorE 11 chunks (fp8 stt runs 1x) ~72us, PE 150K cycles ~63us.

Host: concatenates q over cores, per-class exact fp64 softmax over the
64 nearest candidates, combo -> -(1/sigma2)(u - s*combo).
"""

import numpy as np

N, CH, HH, WW = 50000, 3, 32, 32
D = CH * HH * WW        # 3072
K = 10
NCORES = 8
NSHARD = N // NCORES    # 6250
P = 128
NCHUNK = D // P         # 24 feature chunks
NH = 2                  # sample halves per core
HWID = NSHARD // NH     # 3125
FREE = 512              # PSUM matmul slice width (fp32)
NSL = (HWID + FREE - 1) // FREE   # 7 slices (6x512 + 53)
SUP = 2                 # chunks per DMA
TOPK = 64               # host re-rank candidates per class

# chunk -> engine split, interleaved so the three engines pipeline:
# ScalarE (fused Square(x+b), 153.6 G/s) 11, VectorE (stt, fp8 1x,
# 122.9 G/s) 9, GpSimd (stt, ~62 G/s) 4
SCALAR_CHUNKS = frozenset(range(0, 22, 2))          # 11 even chunks
GPSIMD_CHUNKS = frozenset({5, 11, 17, 23})          # 4 chunks

_NC_CACHE = {}
LAST_RESULTS = None


def _build_nc():
    from contextlib import ExitStack

    import concourse.bacc as bacc
    import concourse.bass as bass
    import concourse.tile as tile
    from concourse import mybir

    f32 = mybir.dt.float32
    bf16 = mybir.dt.bfloat16
    f8 = mybir.dt.float8e4
    Alu = mybir.AluOpType
    Act = mybir.ActivationFunctionType

    nc = bacc.Bacc("TRN2", name="knn_q_score")

    x_d = nc.dram_tensor("xt", [D, NSHARD], f8, kind="ExternalInput")
    bs_d = nc.dram_tensor("nbs", [P, NCHUNK], f32, kind="ExternalInput")
    bd_d = nc.dram_tensor("nbd", [P, NCHUNK], f32, kind="ExternalInput")
    q_d = nc.dram_tensor("q_out", [1, NSHARD], f32, kind="ExternalOutput")

    with ExitStack() as ctx:
        tc = ctx.enter_context(tile.TileContext(nc))
        singles = ctx.enter_context(tc.tile_pool(name="singles", bufs=1))
        xpool = ctx.enter_context(tc.tile_pool(name="xpool", bufs=4))
        sqpool = ctx.enter_context(tc.tile_pool(name="sqpool", bufs=4))
        qpool = ctx.enter_context(tc.tile_pool(name="qpool", bufs=2))
        pspool = ctx.enter_context(tc.tile_pool(name="ps", bufs=1, space="PSUM"))

        bs_sb = singles.tile([P, NCHUNK], f32, tag="bs")
        nc.sync.dma_start(out=bs_sb, in_=bs_d[:, :])
        bd_sb = singles.tile([P, NCHUNK], f32, tag="bd")
        nc.sync.dma_start(out=bd_sb, in_=bd_d[:, :])
        ones_col = singles.tile([P, 1], bf16, tag="ones")
        nc.vector.memset(ones_col, 1.0)

        ps = [
            pspool.tile([1, FREE], f32, tag=f"q{s}", name=f"ps{s}")
            for s in range(NSL)
        ]

        for h in range(NH):
            for g in range(NCHUNK // SUP):
                xt = xpool.tile([P, SUP, HWID], f8, tag="xt", name=f"xt{h}_{g}")
                src = bass.AP(
                    tensor=x_d[:].tensor,
                    offset=(g * SUP * P) * NSHARD + h * HWID,
                    ap=[[NSHARD, P], [NSHARD * P, SUP], [1, HWID]],
                )
                nc.sync.dma_start(out=xt, in_=src)
                for k in range(SUP):
                    ct = g * SUP + k
                    x_c = xt[:, k, :]
                    sq = sqpool.tile([P, HWID], bf16, tag="sq", name=f"sq{h}_{ct}")
                    if ct in SCALAR_CHUNKS:
                        nc.scalar.activation(
                            out=sq,
                            in_=x_c,
                            func=Act.Square,
                            bias=bs_sb[:, ct : ct + 1],
                            scale=1.0,
                        )
                    elif ct in GPSIMD_CHUNKS:
                        # rows pre-shifted to (x - c) on host: plain square
                        # (Pool codegen rejects per-partition scalar operands)
                        nc.gpsimd.tensor_mul(sq, x_c, x_c)
                    else:
                        nc.vector.scalar_tensor_tensor(
                            out=sq,
                            in0=x_c,
                            scalar=bd_sb[:, ct : ct + 1],
                            op0=Alu.add,
                            in1=x_c,
                            op1=Alu.mult,
                        )
                    first, last = (ct == 0), (ct == NCHUNK - 1)
                    for s in range(NSL):
                        w = min(FREE, HWID - s * FREE)
                        nc.tensor.matmul(
                            ps[s][:, :w],
                            ones_col[:, :],
                            sq[:, s * FREE : s * FREE + w],
                            start=first,
                            stop=last,
                        )
            # drain PSUM -> SBUF (DMA cannot read PSUM), then one DMA out
            qrow = qpool.tile([1, HWID], f32, tag="qrow", name=f"qrow{h}")
            for s in range(NSL):
                w = min(FREE, HWID - s * FREE)
                dst = qrow[:, s * FREE : s * FREE + w]
                if s % 2 == 0:
                    nc.scalar.copy(out=dst, in_=ps[s][:, :w])
                else:
                    nc.vector.tensor_copy(dst, ps[s][:, :w])
            nc.sync.dma_start(out=q_d[:, h * HWID : (h + 1) * HWID], in_=qrow)

    nc.finalize()
    return nc


def kernel(u, x_data, y, alpha_bar, t):
    import ml_dtypes
    from concourse.bass_utils import run_bass_kernel_spmd

    u = np.asarray(u, dtype=np.float32)
    x_data = np.asarray(x_data, dtype=np.float32)
    y = np.asarray(y)
    alpha_bar = np.asarray(alpha_bar, dtype=np.float32)
    ti = int(np.asarray(t))

    a_bar = float(alpha_bar[ti])
    s = float(np.sqrt(a_bar))
    sigma2 = 1.0 - a_bar

    if "nc" not in _NC_CACHE:
        _NC_CACHE["nc"] = _build_nc()
    nc = _NC_CACHE["nc"]

    x_flat = x_data.reshape(N, D)
    u_flat = np.ascontiguousarray(u.reshape(D)).astype(np.float64)
    c = (u_flat / s).astype(np.float32)               # (D,)
    nbs = np.ascontiguousarray((-c).reshape(NCHUNK, P).T)        # [P, NCHUNK]
    nbd = np.ascontiguousarray((-2.0 * c).reshape(NCHUNK, P).T)  # [P, NCHUNK]

    # GpSimd chunks get host-pre-shifted rows (x - c) so the device op is a
    # plain self-multiply; those values reach ~25 so the whole tensor uses
    # e4m3 (range 240).  Remaining chunks stay raw x (exact fp32 bias on
    # device), only paying e4m3's 3.6% quantization on |x| ~ 1.
    cg = np.zeros(D, dtype=np.float32)
    for ct in GPSIMD_CHUNKS:
        cg[ct * P : (ct + 1) * P] = c[ct * P : (ct + 1) * P]
    x8 = (x_flat - cg[None, :]).astype(ml_dtypes.float8_e4m3)
    in_maps = []
    for i in range(NCORES):
        xt = np.ascontiguousarray(x8[i * NSHARD : (i + 1) * NSHARD].T)
        in_maps.append({"xt": xt, "nbs": nbs, "nbd": nbd})

    import os

    trace = os.environ.get("KNN_TRACE", "0") == "1"
    res = run_bass_kernel_spmd(
        nc, in_maps, core_ids=list(range(NCORES)), trace=trace
    )
    global LAST_RESULTS
    LAST_RESULTS = res

    q = np.concatenate([r["q_out"].reshape(-1) for r in res.results])  # (N,)

    # host re-rank: exact fp64 softmax over the TOPK nearest rows per class
    combo = np.zeros((K, D), dtype=np.float64)
    for cls in range(K):
        idx = np.flatnonzero(y == cls)
        if len(idx) > TOPK:
            sel = np.argpartition(q[idx], TOPK)[:TOPK]
            idx = idx[sel]
        xr = x_flat[idx].astype(np.float64)           # (k, D)
        d = u_flat[None, :] - s * xr
        logits = -(0.5 / sigma2) * np.sum(d * d, axis=1)
        logits -= logits.max()
        w = np.exp(logits)
        w /= w.sum()
        combo[cls] = w @ xr
    result = -(1.0 / sigma2) * (u_flat[None, :] - s * combo)
    return result.astype(np.float32).reshape(K, 1, CH, HH, WW)


# revision 17
# speedup vs baseline: 1.1487x; 1.1487x over previous
"""Trainium2 kernel for the conditional optimal diffusion score
(per-class masked-softmax RBF regression over the dataset).

Math (see reference): for query u, dataset x (N,D), labels y (N,):
    logit_n = -(0.5/sigma2) * ||u - s*x_n||^2,  s = sqrt(alpha_bar[t])
            = -(s^2/(2*sigma2)) * ||x_n - u/s||^2
so ranking samples by logit (descending) == ranking by
    q_n = ||x_n - c||^2,  c = u/s   (ascending).
The per-class softmax at this noise level is extremely concentrated
(logit std across samples ~17), so the exact score is a tiny weighted
sum over the few nearest neighbours per class.  The device therefore
only needs q_n to ~1-logit accuracy for CANDIDATE SELECTION; the host
re-ranks the top-64 rows per class exactly in fp64.

Device strategy (per core, shard = 6250 rows of x):
  x is streamed TRANSPOSED (partitions = feature dim d, free = sample n)
  in fp8 e3m4 (1 byte/elem -> 4x less HBM traffic than fp32).  For each
  128-row feature chunk ct (24 per core):
    ScalarE chunks: sq = Square(x + b),   b = -c  (bias is per-partition)
    VectorE chunks: sq = (x + b2) * x,    b2 = -2c  (fused stt; differs
       from Square chunks only by a per-chunk constant sum(c_d^2), which
       is sample-independent and thus ranking-safe)
  and the 128-partition reduction q += ones^T @ sq runs on the otherwise
  idle PE array into PSUM (ones is a [128,1] stationary -> ~1 cycle
  weight load, 1 cycle/row streaming).
  The 6250 sample columns are processed in 2 halves of 3125 so the
  per-half PSUM accumulators (7 banks of [1,512] fp32) fit.

Engine budget per core: DMA 19.2MB fp8 ~54us, ScalarE 13 chunks ~68us,
VectorE 11 chunks (fp8 stt runs 1x) ~72us, PE 150K cycles ~63us.

Host: concatenates q over cores, per-class exact fp64 softmax over the
64 nearest candidates, combo -> -(1/sigma2)(u - s*combo).
"""

import numpy as np

N, CH, HH, WW = 50000, 3, 32, 32
D = CH * HH * WW        # 3072
K = 10
NCORES = 8
NSHARD = N // NCORES    # 6250
P = 128
NCHUNK = D // P         # 24 feature chunks
NH = 2                  # sample halves per core
HWID = NSHARD // NH     # 3125
FREE = 512              # PSUM matmul slice width (fp32)
NSL = (HWID + FREE - 1) // FREE   # 7 slices (6x512 + 53)
SUP = 2                 # chunks per DMA
TOPK = 64               # host re-rank candidates per class

# chunk -> engine split, interleaved so the engines pipeline.
# ScalarE (fused Square(x+b), 153.6 G/s any dtype) reads fp8 straight;
# VectorE chunks are DMA-upcast fp8->bf16 in flight (SWDGE cast) so the
# stt runs in 2x mode (245.8 G/s).  GpSimd elementwise is NOT used: its
# SBUF port is shared with VectorE and measurably stalls it.
SCALAR_CHUNKS = frozenset(range(0, 22, 2))          # 11 even chunks
GPSIMD_CHUNKS = frozenset()                         # disabled (port contention)
PSW = NSL * FREE                                    # spanning PSUM tile width

_NC_CACHE = {}
LAST_RESULTS = None


def _build_nc():
    from contextlib import ExitStack

    import concourse.bacc as bacc
    import concourse.bass as bass
    import concourse.tile as tile
    from concourse import mybir

    f32 = mybir.dt.float32
    bf16 = mybir.dt.bfloat16
    f8 = mybir.dt.float8e4
    Alu = mybir.AluOpType
    Act = mybir.ActivationFunctionType

    nc = bacc.Bacc("TRN2", name="knn_q_score")

    x_d = nc.dram_tensor("xt", [D, NSHARD], f8, kind="ExternalInput")
    bs_d = nc.dram_tensor("nbs", [P, NCHUNK], f32, kind="ExternalInput")
    q_d = nc.dram_tensor("q_out", [1, NSHARD], f32, kind="ExternalOutput")

    with ExitStack() as ctx:
        tc = ctx.enter_context(tile.TileContext(nc))
        singles = ctx.enter_context(tc.tile_pool(name="singles", bufs=1))
        xpool = ctx.enter_context(tc.tile_pool(name="xpool", bufs=4))
        sqpool = ctx.enter_context(tc.tile_pool(name="sqpool", bufs=4))
        qpool = ctx.enter_context(tc.tile_pool(name="qpool", bufs=2))
        pspool = ctx.enter_context(tc.tile_pool(name="ps", bufs=1, space="PSUM"))

        bs_sb = singles.tile([P, NCHUNK], f32, tag="bs")
        nc.sync.dma_start(out=bs_sb, in_=bs_d[:, :])
        ones_col = singles.tile([P, 1], bf16, tag="ones")
        nc.vector.memset(ones_col, 1.0)

        # one PSUM tile spanning NSL banks; each matmul writes one bank slice
        ps = pspool.tile([1, PSW], f32, tag="q", name="ps")

        for h in range(NH):
            for ct in range(NCHUNK):
                src = bass.AP(
                    tensor=x_d[:].tensor,
                    offset=(ct * P) * NSHARD + h * HWID,
                    ap=[[NSHARD, P], [1, HWID]],
                )
                if ct in SCALAR_CHUNKS:
                    x_c = xpool.tile([P, HWID], f8, tag="xts", name=f"xt{h}_{ct}")
                    nc.sync.dma_start(out=x_c, in_=src)
                else:
                    # SWDGE cast-DMA: fp8 in HBM -> bf16 in SBUF
                    x_c = xpool.tile([P, HWID], bf16, tag="xtv", name=f"xt{h}_{ct}")
                    nc.gpsimd.dma_start(out=x_c, in_=src)
                sq = sqpool.tile([P, HWID], bf16, tag="sq", name=f"sq{h}_{ct}")
                if ct in SCALAR_CHUNKS:
                    nc.scalar.activation(
                        out=sq,
                        in_=x_c,
                        func=Act.Square,
                        bias=bs_sb[:, ct : ct + 1],
                        scale=1.0,
                    )
                else:
                    # two-instruction square: TS-add runs 4x, TT-mult 2x on
                    # bf16 (the fused stt only has a 1x uop)
                    tmp = sqpool.tile(
                        [P, HWID], bf16, tag="tmp", name=f"tmp{h}_{ct}"
                    )
                    nc.vector.tensor_scalar(
                        tmp, x_c, bs_sb[:, ct : ct + 1], None, Alu.add
                    )
                    nc.vector.tensor_tensor(sq, tmp, tmp, Alu.mult)
                first, last = (ct == 0), (ct == NCHUNK - 1)
                for s in range(NSL):
                    w = min(FREE, HWID - s * FREE)
                    nc.tensor.matmul(
                        ps[:, s * FREE : s * FREE + w],
                        ones_col[:, :],
                        sq[:, s * FREE : s * FREE + w],
                        start=first,
                        stop=last,
                    )
            # slice columns land contiguously in the spanning PSUM tile:
            # one copy (DMA cannot read PSUM), one DMA out
            qrow = qpool.tile([1, HWID], f32, tag="qrow", name=f"qrow{h}")
            nc.scalar.copy(out=qrow, in_=ps[:, :HWID])
            nc.sync.dma_start(out=q_d[:, h * HWID : (h + 1) * HWID], in_=qrow)

    nc.finalize()
    return nc


def kernel(u, x_data, y, alpha_bar, t):
    import ml_dtypes
    from concourse.bass_utils import run_bass_kernel_spmd

    u = np.asarray(u, dtype=np.float32)
    x_data = np.asarray(x_data, dtype=np.float32)
    y = np.asarray(y)
    alpha_bar = np.asarray(alpha_bar, dtype=np.float32)
    ti = int(np.asarray(t))

    a_bar = float(alpha_bar[ti])
    s = float(np.sqrt(a_bar))
    sigma2 = 1.0 - a_bar

    if "nc" not in _NC_CACHE:
        _NC_CACHE["nc"] = _build_nc()
    nc = _NC_CACHE["nc"]

    x_flat = x_data.reshape(N, D)
    u_flat = np.ascontiguousarray(u.reshape(D)).astype(np.float64)
    c = (u_flat / s).astype(np.float32)               # (D,)
    nbs = np.ascontiguousarray((-c).reshape(NCHUNK, P).T)        # [P, NCHUNK]

    x8 = x_flat.astype(ml_dtypes.float8_e4m3)
    in_maps = []
    for i in range(NCORES):
        xt = np.ascontiguousarray(x8[i * NSHARD : (i + 1) * NSHARD].T)
        in_maps.append({"xt": xt, "nbs": nbs})

    import os

    trace = os.environ.get("KNN_TRACE", "0") == "1"
    res = run_bass_kernel_spmd(
        nc, in_maps, core_ids=list(range(NCORES)), trace=trace
    )
    global LAST_RESULTS
    LAST_RESULTS = res

    q = np.concatenate([r["q_out"].reshape(-1) for r in res.results])  # (N,)

    # host re-rank: exact fp64 softmax over the TOPK nearest rows per class
    combo = np.zeros((K, D), dtype=np.float64)
    for cls in range(K):
        idx = np.flatnonzero(y == cls)
        if len(idx) > TOPK:
            sel = np.argpartition(q[idx], TOPK)[:TOPK]
            idx = idx[sel]
        xr = x_flat[idx].astype(np.float64)           # (k, D)
        d = u_flat[None, :] - s * xr
        logits = -(0.5 / sigma2) * np.sum(d * d, axis=1)
        logits -= logits.max()
        w = np.exp(logits)
        w /= w.sum()
        combo[cls] = w @ xr
    result = -(1.0 / sigma2) * (u_flat[None, :] - s * combo)
    return result.astype(np.float32).reshape(K, 1, CH, HH, WW)
